# revision 6
# baseline (speedup 1.0000x reference)
"""GNN message passing (lin1+relu -> SAGEConv(mean) -> relu -> lin2) on 8 trn2 cores.

Sharding: destination nodes are partitioned across the 8 NeuronCores (12500
each).  Every core holds the full lin1 activation table h in SBUF in a
feature-transposed layout split into 8 src-range groups (one per GPSIMD Q7
core).  Edges are bucketed host-side by (dst-core, src-group) and sorted by
dst; per-edge messages are produced with on-chip ap_gather, reduced per dst
node with a prefix scan + segment-boundary gather, combined across groups on
the PE, and fed through the remaining dense layers.
"""

import numpy as np

F = 16
F2 = 32


def make_cfg(n_nodes, n_edges, ncores=8, nr=8):
    g = 8
    assert n_nodes % ncores == 0 and n_nodes % g == 0
    nv = n_nodes // ncores  # dst nodes per core
    nsrc = n_nodes // g  # src nodes per gpsimd group table
    assert nsrc + 16 <= 32768
    vr = -(-nv // nr)  # v-range width per round
    vr = -(-vr // 16) * 16
    assert vr % 16 == 0 and nr * vr >= nv and nr % 2 == 0
    return dict(
        n_nodes=n_nodes, n_edges=n_edges, ncores=ncores, g=g, nr=nr,
        nv=nv, nsrc=nsrc, vr=vr, tab=nsrc + 16, sent=nsrc,
    )


CFG = make_cfg(100000, 3200000)


def preprocess(x, edge_index, cfg):
    """Shard/reorder inputs host-side. Returns (per-core array dict, CAP)."""
    NC, G, NR = cfg["ncores"], cfg["g"], cfg["nr"]
    NV, NSRC, VR, SENT = cfg["nv"], cfg["nsrc"], cfg["vr"], cfg["sent"]
    E = cfg["n_edges"]

    src = np.asarray(edge_index[0], dtype=np.int64)
    dst = np.asarray(edge_index[1], dtype=np.int64)
    x = np.asarray(x, dtype=np.float32)

    core = dst // NV
    grp = src // NSRC
    dstl = dst - core * NV
    rnd = dstl // VR
    bucket = ((core * G + grp) * NR + rnd).astype(np.int64)

    order = np.lexsort((dstl, bucket))
    bucket_s = bucket[order]
    srcl_s = (src[order] - grp[order] * NSRC).astype(np.int32)

    nbuckets = NC * G * NR
    bcnt = np.bincount(bucket_s, minlength=nbuckets)
    # multiple of 32 so each round's int16 idx slice is 4-byte aligned
    CAP = int(max(32, -(-int(bcnt.max()) // 32) * 32))

    # position of each edge within its bucket
    starts = np.zeros(nbuckets, dtype=np.int64)
    np.cumsum(bcnt[:-1], out=starts[1:])
    within = np.arange(E, dtype=np.int64) - starts[bucket_s]

    # padded per-bucket src-local index lists [NC, G, NR, CAP]
    idx_arr = np.full((nbuckets, CAP), SENT, dtype=np.int16)
    idx_arr[bucket_s, within] = srcl_s.astype(np.int16)
    idx_arr = idx_arr.reshape(NC, G, NR, CAP)

    def wrap2(a):
        # [NC, G, NR, L] -> [NC, 128, NR*(L//16)]; list elem j of group g at
        # partition 16*g + j%16, col j//16 (ap_gather wrapped layout), rounds
        # concatenated along the free dim.
        nc_, g_, nr_, L = a.shape
        w = a.reshape(nc_, g_, nr_, L // 16, 16).transpose(0, 2, 1, 4, 3)
        # [NC, NR, G, 16, L//16] -> partitions 16g+j%16
        w = w.reshape(nc_, nr_, g_ * 16, L // 16)
        return np.ascontiguousarray(w.transpose(0, 2, 1, 3).reshape(nc_, g_ * 16, nr_ * (L // 16)))

    srcidx = wrap2(idx_arr)

    # per-(core, group, node) counts -> within-round inclusive cumsum
    cnt_kgv = np.bincount((core * G + grp) * NV + dstl, minlength=NC * G * NV)
    cnt_kgv = cnt_kgv.reshape(NC, G, NV)
    cnt_pad = np.zeros((NC, G, NR * VR), dtype=np.int64)
    cnt_pad[:, :, :NV] = cnt_kgv
    cnt_rounds = cnt_pad.reshape(NC, G, NR, VR)
    e_idx = np.cumsum(cnt_rounds, axis=3)
    assert int(e_idx[..., -1].max()) <= CAP
    bndidx = wrap2(e_idx.astype(np.int16))

    # total per-node counts, spread layout [NC, 128, VR] (partition 16r+f)
    cnt_total = cnt_kgv.sum(axis=1).astype(np.float32)  # [NC, NV]
    ct = np.zeros((NC, NR * VR), dtype=np.float32)
    ct[:, :NV] = cnt_total
    cnt_spread = np.repeat(ct.reshape(NC, NR, 1, VR), 16, axis=2).reshape(NC, 128, VR)

    # x tables: xT_all [128, TAB] same for all cores; xT_dst [NC, 128, VR]
    TAB = cfg["tab"]
    xt = np.zeros((G, F, TAB), dtype=np.float32)
    xt[:, :, :NSRC] = x.reshape(G, NSRC, F).transpose(0, 2, 1)
    xT_all = xt.reshape(128, TAB)

    xd = np.zeros((NC, NR, VR, F), dtype=np.float32)
    xd.reshape(NC, NR * VR, F)[:, :NV] = x.reshape(NC, NV, F)
    xT_dst = np.ascontiguousarray(xd.transpose(0, 1, 3, 2).reshape(NC, 128, VR))

    per_core = []
    for k in range(NC):
        per_core.append(dict(
            xT_all=np.ascontiguousarray(xT_all),
            xT_dst=xT_dst[k],
            srcidx=srcidx[k],
            bndidx=bndidx[k],
            cnt=cnt_spread[k],
        ))
    return per_core, CAP


def make_weights(lin1_w, lin1_b, sage_wl, sage_bl, sage_wr, lin2_w, lin2_b, cfg):
    G = cfg["g"]
    W1blk = np.zeros((128, 128), dtype=np.float32)
    Wr_blk = np.zeros((128, 128), dtype=np.float32)
    b1col = np.zeros((128, 1), dtype=np.float32)
    I16lo = np.zeros((128, 32), dtype=np.float32)
    I16hi = np.zeros((128, 32), dtype=np.float32)
    for c in range(G):
        W1blk[16 * c:16 * c + 16, 16 * c:16 * c + 16] = lin1_w
        Wr_blk[16 * c:16 * c + 16, 16 * c:16 * c + 16] = sage_wr
        b1col[16 * c:16 * c + 16, 0] = lin1_b
        I16lo[16 * c:16 * c + 16, 0:16] = np.eye(16, dtype=np.float32)
        I16hi[16 * c:16 * c + 16, 16:32] = np.eye(16, dtype=np.float32)
    Wl2 = np.zeros((32, 32), dtype=np.float32)
    W2b = np.zeros((32, 64), dtype=np.float32)
    bl2 = np.zeros((128, 1), dtype=np.float32)
    b2st = np.zeros((64, 1), dtype=np.float32)
    for h in range(2):
        Wl2[16 * h:16 * h + 16, 16 * h:16 * h + 16] = sage_wl
        W2b[16 * h:16 * h + 16, 32 * h:32 * h + 32] = lin2_w
        b2st[32 * h:32 * h + 32, 0] = lin2_b
    for c in range(G):
        bl2[16 * c:16 * c + 16, 0] = sage_bl
    return dict(
        W1blk=W1blk, Wr_blk=Wr_blk, b1col=b1col, I16lo=I16lo, I16hi=I16hi,
        Wl2=Wl2, W2b=W2b, bl2=bl2, b2st=b2st,
    )


def build_program(cfg, CAP, _skip=(), _loop_n=None):
    import concourse.bacc as bacc
    import concourse.tile as tile
    import concourse.mybir as mybir

    NR, VR, TAB, SENT = cfg["nr"], cfg["vr"], cfg["tab"], cfg["sent"]
    NCORES = cfg["ncores"]
    dt = mybir.dt
    AF = mybir.ActivationFunctionType
    OP = mybir.AluOpType
    CC = CAP // 16
    VC = VR // 16

    nc = bacc.Bacc("TRN2", target_bir_lowering=False, debug=False,
                   num_devices=NCORES)

    def inp(name, shape, dtype):
        return nc.dram_tensor(name, shape, dtype, kind="ExternalInput").ap()

    xT_all = inp("xT_all", [128, TAB], dt.float32)
    xT_dst = inp("xT_dst", [128, VR], dt.float32)
    srcidx = inp("srcidx", [128, NR * CC], dt.int16)
    bndidx = inp("bndidx", [128, NR * VC], dt.int16)
    cnt = inp("cnt", [128, VR], dt.float32)
    W1blk = inp("W1blk", [128, 128], dt.float32)
    Wr_blk = inp("Wr_blk", [128, 128], dt.float32)
    b1col = inp("b1col", [128, 1], dt.float32)
    I16lo = inp("I16lo", [128, 32], dt.float32)
    I16hi = inp("I16hi", [128, 32], dt.float32)
    Wl2 = inp("Wl2", [32, 32], dt.float32)
    bl2 = inp("bl2", [128, 1], dt.float32)
    W2b = inp("W2b", [32, 64], dt.float32)
    b2st = inp("b2st", [64, 1], dt.float32)
    outT = nc.dram_tensor("outT", [2 * F2, (NR // 2) * VR], dt.float32,
                          kind="ExternalOutput").ap()

    def sb(name, shape, dtype):
        return nc.alloc_sbuf_tensor(name, list(shape), dtype).ap()

    htab = sb("htab", [128, TAB], dt.float32)
    hwr = sb("hwr", [128, VR], dt.float32)
    recip = sb("recip", [128, VR], dt.float32)
    xdst_sb = sb("xdst_sb", [128, VR], dt.float32)
    srcidx_sb = sb("srcidx_sb", [128, NR * CC], dt.int16)
    bndidx_sb = sb("bndidx_sb", [128, NR * VC], dt.int16)
    # msgs/ebuf are double-buffered by round parity so round r+1's gather
    # (Pool) can run while round r's scan/subtract (DVE) still read them.
    msgs2 = [sb("msgs0", [128, CAP], dt.float32),
             sb("msgs1", [128, CAP], dt.float32)]
    scanT = sb("scanT", [128, CAP + 1], dt.float32)
    ebuf2 = [sb("ebuf0", [128, VR + 1], dt.float32),
             sb("ebuf1", [128, VR + 1], dt.float32)]
    diff_a = sb("diff_a", [128, VR], dt.float32)
    diff_b = sb("diff_b", [128, VR], dt.float32)
    w1_sb = sb("w1_sb", [128, 128], dt.float32)
    wr_sb = sb("wr_sb", [128, 128], dt.float32)
    b1_sb = sb("b1_sb", [128, 1], dt.float32)
    i16lo_sb = sb("i16lo_sb", [128, 32], dt.float32)
    i16hi_sb = sb("i16hi_sb", [128, 32], dt.float32)
    wl2_sb = sb("wl2_sb", [32, 32], dt.float32)
    bl2_sb = sb("bl2_sb", [128, 1], dt.float32)
    w2b_sb = sb("w2b_sb", [32, 64], dt.float32)
    b2st_sb = sb("b2st_sb", [64, 1], dt.float32)

    LIN1_CHUNK = 512
    FCW = min(512, VR)
    n_fc = -(-VR // FCW)

    import contextlib
    with tile.TileContext(nc) as tc:
        loop_cm = tc.For_i(0, _loop_n, 1) if _loop_n else contextlib.nullcontext()
        with loop_cm, \
             tc.tile_pool(name="stage", bufs=2) as stage_pool, \
             tc.tile_pool(name="psum", bufs=2, space="PSUM") as psum_pool, \
             tc.tile_pool(name="psum_s", bufs=2, space="PSUM") as psum_s_pool:

            # ---- load small inputs ----
            nc.sync.dma_start(out=w1_sb, in_=W1blk)
            nc.sync.dma_start(out=wr_sb, in_=Wr_blk)
            nc.sync.dma_start(out=b1_sb, in_=b1col)
            nc.sync.dma_start(out=i16lo_sb, in_=I16lo)
            nc.sync.dma_start(out=i16hi_sb, in_=I16hi)
            nc.sync.dma_start(out=wl2_sb, in_=Wl2)
            nc.sync.dma_start(out=bl2_sb, in_=bl2)
            nc.sync.dma_start(out=w2b_sb, in_=W2b)
            nc.sync.dma_start(out=b2st_sb, in_=b2st)
            nc.sync.dma_start(out=srcidx_sb, in_=srcidx)
            nc.sync.dma_start(out=bndidx_sb, in_=bndidx)
            nc.sync.dma_start(out=diff_b, in_=cnt)
            nc.sync.dma_start(out=xdst_sb, in_=xT_dst)

            # ---- lin1 into the transposed gather table ----
            for c0 in ([] if "lin1" in _skip else range(0, TAB, LIN1_CHUNK)):
                w = min(LIN1_CHUNK, TAB - c0)
                xst = stage_pool.tile([128, LIN1_CHUNK], dt.float32, tag="xst")
                nc.sync.dma_start(out=xst[:, :w], in_=xT_all[:, c0:c0 + w])
                pt = psum_pool.tile([128, LIN1_CHUNK], dt.float32, tag="p128")
                nc.tensor.matmul(out=pt[:, :w], lhsT=w1_sb, rhs=xst[:, :w],
                                 start=True, stop=True)
                nc.scalar.activation(out=htab[:, c0:c0 + w], in_=pt[:, :w],
                                     func=AF.Relu, bias=b1_sb[:, 0:1], scale=1.0)
            nc.vector.memset(htab[:, SENT:TAB], 0)

            # ---- dst shard: hwr = relu(lin1(x_dst)) @ Wr, spread layout ----
            for i in ([] if "hwr" in _skip else range(n_fc)):
                c0 = i * FCW
                w = min(FCW, VR - c0)
                pt = psum_pool.tile([128, LIN1_CHUNK], dt.float32, tag="p128")
                nc.tensor.matmul(out=pt[:, :w], lhsT=w1_sb,
                                 rhs=xdst_sb[:, c0:c0 + w], start=True, stop=True)
                ht = stage_pool.tile([128, FCW], dt.float32, tag="ht")
                nc.scalar.activation(out=ht[:, :w], in_=pt[:, :w],
                                     func=AF.Relu, bias=b1_sb[:, 0:1], scale=1.0)
                pt2 = psum_pool.tile([128, LIN1_CHUNK], dt.float32, tag="p128")
                nc.tensor.matmul(out=pt2[:, :w], lhsT=wr_sb, rhs=ht[:, :w],
                                 start=True, stop=True)
                nc.vector.tensor_copy(out=hwr[:, c0:c0 + w], in_=pt2[:, :w])

            # ---- 1 / max(cnt, 1)  (cnt landed in diff_b) ----
            nc.vector.tensor_scalar_max(diff_a, diff_b, 1.0)
            nc.vector.reciprocal(recip, diff_a)

            # constant zero columns (never overwritten by scan/bgather)
            nc.vector.memset(scanT[:, 0:1], 0)
            nc.vector.memset(ebuf2[0][:, 0:1], 0)
            nc.vector.memset(ebuf2[1][:, 0:1], 0)

            def emit_mgather(r):
                nc.gpsimd.ap_gather(
                    out_ap=msgs2[r % 2], in_ap=htab,
                    idxs_ap=srcidx_sb[:, r * CC:(r + 1) * CC],
                    channels=128, num_elems=TAB, d=1, num_idxs=CAP)

            def emit_round_tail(r, dbuf):
                m = msgs2[r % 2]
                e = ebuf2[r % 2]
                if "scan" not in _skip:
                    nc.vector.tensor_tensor_scan(
                        out=scanT[:, 1:CAP + 1], data0=m, data1=m,
                        initial=0.0, op0=OP.add, op1=OP.bypass)
                if "bgather" not in _skip:
                    nc.gpsimd.ap_gather(
                        out_ap=e[:, 1:VR + 1], in_ap=scanT,
                        idxs_ap=bndidx_sb[:, r * VC:(r + 1) * VC],
                        channels=128, num_elems=CAP + 1, d=1, num_idxs=VR)
                nc.vector.tensor_tensor(out=dbuf, in0=e[:, 1:VR + 1],
                                        in1=e[:, 0:VR], op=OP.subtract)

            # ---- software-pipelined rounds: the Pool stream interleaves
            # mgather(r+1) before bgather(r) so the in-order GPSIMD engine
            # never stalls on the DVE scan of round r. ----
            if "edge" not in _skip and "gather" not in _skip:
                emit_mgather(0)
            for R in range(NR // 2):
                for h, dbuf in ([] if "edge" in _skip else ((0, diff_a), (1, diff_b))):
                    r = 2 * R + h
                    if "gather" not in _skip and r + 1 < NR:
                        emit_mgather(r + 1)
                    emit_round_tail(r, dbuf)
                for i in ([] if "final" in _skip else range(n_fc)):
                    c0 = i * FCW
                    w = min(FCW, VR - c0)
                    pc = psum_s_pool.tile([32, FCW], dt.float32, tag="pc")
                    nc.tensor.matmul(out=pc[:, :w], lhsT=i16lo_sb,
                                     rhs=diff_a[:, c0:c0 + w], start=True, stop=False)
                    nc.tensor.matmul(out=pc[:, :w], lhsT=i16hi_sb,
                                     rhs=diff_b[:, c0:c0 + w], start=False, stop=True)
                    aggst = stage_pool.tile([32, FCW], dt.float32, tag="aggst")
                    nc.vector.tensor_tensor(
                        out=aggst[:, :w], in0=pc[:, :w],
                        in1=recip[32 * R:32 * R + 32, c0:c0 + w], op=OP.mult)
                    pz = psum_s_pool.tile([32, FCW], dt.float32, tag="pz")
                    nc.tensor.matmul(out=pz[:, :w], lhsT=wl2_sb,
                                     rhs=aggst[:, :w], start=True, stop=True)
                    zpre = stage_pool.tile([32, FCW], dt.float32, tag="zpre")
                    nc.vector.scalar_tensor_tensor(
                        out=zpre[:, :w], in0=pz[:, :w],
                        scalar=bl2_sb[32 * R:32 * R + 32, 0:1],
                        in1=hwr[32 * R:32 * R + 32, c0:c0 + w],
                        op0=OP.add, op1=OP.add)
                    zt = stage_pool.tile([32, FCW], dt.float32, tag="zt")
                    nc.vector.tensor_scalar_max(zt[:, :w], zpre[:, :w], 0.0)
                    po = psum_s_pool.tile([64, FCW], dt.float32, tag="po")
                    nc.tensor.matmul(out=po[:, :w], lhsT=w2b_sb, rhs=zt[:, :w],
                                     start=True, stop=True)
                    ot = stage_pool.tile([64, FCW], dt.float32, tag="ot")
                    nc.vector.tensor_scalar_add(ot[:, :w], po[:, :w],
                                                b2st_sb[:, 0:1])
                    nc.sync.dma_start(out=outT[:, R * VR + c0:R * VR + c0 + w],
                                      in_=ot[:, :w])

    nc.compile()
    return nc


def run_kernel(x, edge_index, lin1_w, lin1_b, sage_wl, sage_bl, sage_wr,
               lin2_w, lin2_b, cfg=None, trace=False):
    from concourse import bass_utils

    if cfg is None:
        cfg = CFG
    per_core, CAP = preprocess(x, edge_index, cfg)
    weights = make_weights(lin1_w, lin1_b, sage_wl, sage_bl, sage_wr,
                           lin2_w, lin2_b, cfg)
    in_maps = [dict(pc, **weights) for pc in per_core]
    nc = build_program(cfg, CAP)
    res = bass_utils.run_bass_kernel_spmd(
        nc, in_maps, core_ids=list(range(cfg["ncores"])), trace=trace)

    NV, NR, VR = cfg["nv"], cfg["nr"], cfg["vr"]
    out = np.empty((cfg["n_nodes"], F2), dtype=np.float32)
    for k in range(cfg["ncores"]):
        ot = res.results[k]["outT"]  # [64, (NR//2)*VR]
        full = np.empty((NR * VR, F2), dtype=np.float32)
        for R in range(NR // 2):
            for hh in range(2):
                full[(2 * R + hh) * VR:(2 * R + hh + 1) * VR, :] = \
                    ot[32 * hh:32 * hh + 32, R * VR:(R + 1) * VR].T
        out[NV * k:NV * (k + 1), :] = full[:NV]
    return out, res


def kernel(**inputs):
    out, _ = run_kernel(**inputs)
    return out

